# revision 19
# baseline (speedup 1.0000x reference)
"""Trainium2 Bass kernel for a decoder block (LN -> MHA -> LN -> FFN).

Sharding: heads across the 8 cores for attention (2 heads/core), tokens
across cores for dense/LN2/FFN (512 tokens/core), connected by an
AllToAll of the softmax-normalized ctx in bf16 — one collective per
batch; the first overlaps batch-1 attention, the second overlaps the
batch-0 half of the dense/FFN phase.

v2 structural changes vs v1:
- AV matmul flipped: probs tiles are the stationary operand, ctx comes
  out token-major [q, 64+1] with the softmax denominator in the last
  column -> per-partition reciprocal + tensor_scalar (kills the
  lane-starved [1,512] reciprocal / broadcast-matmul / big multiply).
- A2A payload is token-major; consumer rebuilds feature-major ctxT with
  16 PE transposes per half (cheap, in a phase where PE has slack).
- Causal mask adds narrowed to the 128-wide diagonal band, both heads
  in one op; exp for both heads in one ACT instruction per k-tile.
- Causal mask applied AFTER exp as a bf16 0/1 band multiply, so the
  DVE leaves the scores->exp critical cycle.
- rstd via DVE fast-rsqrt (bit trick + 2 Newton steps) -> no ACT
  Sqrt/Exp activation-table swaps.
- All weights pre-arranged on the host into partition-contiguous
  layouts (one contiguous chunk per SBUF partition per DMA) to kill
  Sync-queue descriptor-generation cost.
- Phase 2 schedule: ctx0/dense0/LN2-0 -> fc0 -> ctx1/dense1/LN2-1 ->
  fc1 -> merged proj over all 512 tokens (proj weights loaded once);
  fc weights stream while A2A#1 is in flight.
"""

import numpy as np
import ml_dtypes

B, S, D = 2, 2048, 1024
H, DEP = 16, 64
NT = B * S            # 4096 flattened tokens
NCORES = 8
HPC = H // NCORES     # 2 heads per core
TPC = NT // NCORES    # 512 tokens per core
QM = 512              # q-macro / token-macro size
NEG = -1.0e9
AV_LAG = 3            # k-tiles of slack between exp and AV consumption

_cache = {}


def _build_program():
    from contextlib import ExitStack
    import concourse.bacc as bacc
    import concourse.tile as tile
    import concourse.mybir as mybir
    from concourse.masks import make_identity

    dt = mybir.dt
    AF = mybir.ActivationFunctionType
    OP = mybir.AluOpType

    nc = bacc.Bacc("TRN2", target_bir_lowering=False, debug=False,
                   num_devices=NCORES)

    def din(name, shape, dtype=dt.float32):
        return nc.dram_tensor(name, shape, dtype, kind="ExternalInput").ap()

    x_full = din("x_full", [NT, D])
    x_shard = din("x_shard", [TPC, D])
    wqt = din("wqt", [128, 8, 128], dt.bfloat16)
    wkt = din("wkt", [128, 8, 128], dt.bfloat16)
    wvt = din("wvt", [128, 8, 128], dt.bfloat16)
    qb_i = din("qb", [128, 1])
    kb_i = din("kb", [128, 1])
    vb_i = din("vb", [128, 1])
    g1b_i = din("g1b", [128, D])
    b1b_i = din("b1b", [128, D])
    g2b_i = din("g2b", [128, D])
    b2b_i = din("b2b", [128, D])
    dense_wt = din("dense_wt", [128, 8, D], dt.bfloat16)
    fc_wt = din("fc_wt", [4, 128, 8, 8, 128], dt.bfloat16)
    fcb_i = din("fcb", [128, 32])
    proj_wt = din("proj_wt", [4, 128, 32, 256], dt.bfloat16)
    mask_i = din("mask_tri", [128, 2, 128], dt.bfloat16)
    out_sh = nc.dram_tensor("out_shard", [TPC, D], dt.float32,
                            kind="ExternalOutput").ap()

    VAR_SCALE = float(D) / float(D - 1)   # ddof=1 correction
    EPS = 1e-6

    with tile.TileContext(nc) as tc:
        with ExitStack() as es0:
            consts = es0.enter_context(tc.tile_pool(name="consts", bufs=1))
            dram = es0.enter_context(
                tc.tile_pool(name="dram", bufs=1, space="DRAM"))
            ident_bf = consts.tile([128, 128], dt.bfloat16)
            make_identity(nc, ident_bf)
            qb = consts.tile([128, 1], dt.float32)
            kb = consts.tile([128, 1], dt.float32)
            vb = consts.tile([128, 1], dt.float32)
            nc.sync.dma_start(out=qb, in_=qb_i)
            nc.sync.dma_start(out=kb, in_=kb_i)
            nc.sync.dma_start(out=vb, in_=vb_i)
            tri2 = consts.tile([128, 2, 128], dt.bfloat16)
            nc.sync.dma_start(out=tri2, in_=mask_i)

            xnsh_pool = es0.enter_context(tc.tile_pool(name="xnsh", bufs=1))
            xn_sh = xnsh_pool.tile([128, 4, D], dt.float32)
            dense_pool = es0.enter_context(tc.tile_pool(name="densew", bufs=1))
            dense_sb = dense_pool.tile([128, 8, D], dt.bfloat16)

            # batch-mixed shards: core c owns 256 tokens of each batch,
            # a2a payload is token-major [dst, head, tok, d].
            a2a_in = [dram.tile([NCORES, HPC, 256, DEP], dt.bfloat16,
                                name=f"a2a_in{bb}") for bb in range(2)]
            a2a_out = [dram.tile([NCORES, HPC, 256, DEP], dt.bfloat16,
                                 name=f"a2a_out{bb}") for bb in range(2)]

            # ------- phase 1: LN1 + QKV + attention, per 512-token macro ---
            with ExitStack() as es1:
                P = lambda *a, **k: es1.enter_context(tc.tile_pool(*a, **k))
                xt_pool = P(name="xt", bufs=3)
                st_pool = P(name="stats", bufs=3)
                xnT_pool = P(name="xnT", bufs=1)
                qkT_pool = P(name="qkT", bufs=1)
                v_pool = P(name="vtok", bufs=1)
                wq_pool = P(name="wq", bufs=1)
                ps_tr = P(name="ps_tr", bufs=1, space="PSUM")
                ps_sc = P(name="ps_sc", bufs=2, space="PSUM")
                ps_qk = P(name="ps_qk", bufs=1, space="PSUM")
                psctx = P(name="psctx", bufs=2, space="PSUM")
                pr_pool = P(name="probs", bufs=17)
                a2a_pool = P(name="a2asb", bufs=4)

                xn_T = xnT_pool.tile([128, 8, NT], dt.bfloat16)
                q_T = qkT_pool.tile([128, NT], dt.bfloat16)
                k_T = qkT_pool.tile([128, NT], dt.bfloat16)
                vtok = v_pool.tile([128, 32, 130], dt.bfloat16)
                nc.vector.memset(vtok[:, :, 64:65], 1.0)
                nc.vector.memset(vtok[:, :, 129:130], 1.0)

                wq_sb = wq_pool.tile([128, 8, 128], dt.bfloat16)
                wk_sb = wq_pool.tile([128, 8, 128], dt.bfloat16)
                wv_sb = wq_pool.tile([128, 8, 128], dt.bfloat16)
                nc.sync.dma_start(out=wq_sb, in_=wqt)
                nc.sync.dma_start(out=wk_sb, in_=wkt)
                nc.sync.dma_start(out=wv_sb, in_=wvt)

                def rsqrt_dve(out_ap, var_ap, n):
                    """rstd = 1/sqrt(var*VAR_SCALE) entirely on DVE:
                    Quake-III bit trick + two Newton iterations."""
                    v = st_pool.tile([128, n], dt.float32, tag="rsq_v",
                                     bufs=2, name="rsq_v")
                    nc.vector.tensor_scalar_mul(out=v, in0=var_ap,
                                                scalar1=VAR_SCALE)
                    y = out_ap
                    yi = y.bitcast(dt.int32)
                    nc.vector.tensor_scalar(
                        out=yi, in0=v.bitcast(dt.int32), scalar1=1,
                        scalar2=None, op0=OP.logical_shift_right)
                    nc.vector.tensor_scalar(
                        out=yi, in0=yi, scalar1=-1, scalar2=0x5f3759df,
                        op0=OP.mult, op1=OP.add)
                    t2 = st_pool.tile([128, n], dt.float32, tag="rsq_t",
                                      bufs=2, name="rsq_t")
                    for _ in range(2):
                        nc.vector.tensor_tensor(out=t2, in0=y, in1=y,
                                                op=OP.mult)
                        nc.vector.tensor_tensor(out=t2, in0=t2, in1=v,
                                                op=OP.mult)
                        nc.vector.tensor_scalar(
                            out=t2, in0=t2, scalar1=-0.5, scalar2=1.5,
                            op0=OP.mult, op1=OP.add)
                        nc.vector.tensor_tensor(out=y, in0=y, in1=t2,
                                                op=OP.mult)

                def ln_macro(src, base_row):
                    """Stats for 4 consecutive 128-row tiles; rstd via DVE
                    fast-rsqrt. Returns [(x_t, mean, rstd)]."""
                    mv4 = st_pool.tile([128, 4, 2], dt.float32, tag="mv4")
                    xts = []
                    for i in range(4):
                        x_t = xt_pool.tile([128, D], dt.float32, tag="xt",
                                           bufs=4)
                        r0 = base_row + 128 * i
                        nc.sync.dma_start(out=x_t, in_=src[r0:r0 + 128, :])
                        stats = st_pool.tile([128, 2, 6], dt.float32,
                                             tag="bnst")
                        nc.vector.bn_stats(out=stats[:, 0, :],
                                           in_=x_t[:, 0:512])
                        nc.vector.bn_stats(out=stats[:, 1, :],
                                           in_=x_t[:, 512:1024])
                        nc.vector.bn_aggr(out=mv4[:, i, :], in_=stats)
                        xts.append(x_t)
                    rstd4 = st_pool.tile([128, 4], dt.float32, tag="rstd4")
                    rsqrt_dve(rstd4, mv4[:, :, 1], 4)
                    return [(xts[i], mv4[:, i, 0:1], rstd4[:, i:i + 1])
                            for i in range(4)]

                qkv_ps = {}

                def qkv_part(m, which):
                    tok = slice(QM * m, QM * (m + 1))
                    if which < 4:
                        w_sb, bias, dst = ((wq_sb, qb, q_T),
                                           (wk_sb, kb, k_T))[which // 2]
                        if which % 2 == 0:
                            qkv_ps[m] = ps_qk.tile([128, QM], dt.float32,
                                                   tag="qk", name="qkps")
                        ps = qkv_ps[m]
                        for k4 in range(4):
                            kc = 4 * (which % 2) + k4
                            nc.tensor.matmul(ps, w_sb[:, kc, :],
                                             xn_T[:, kc, tok],
                                             start=(kc == 0), stop=(kc == 7))
                        if which % 2 == 1:
                            nc.vector.tensor_scalar_add(dst[:, tok], ps, bias)
                        return
                    ps = ps_qk.tile([128, QM], dt.float32, tag="qk",
                                    name="qkps")
                    for kc in range(8):
                        nc.tensor.matmul(ps, wv_sb[:, kc, :], xn_T[:, kc, tok],
                                         start=(kc == 0), stop=(kc == 7))
                    vst = a2a_pool.tile([128, QM], dt.bfloat16, tag="vst",
                                        bufs=2)
                    nc.vector.tensor_scalar_add(vst, ps, vb)
                    for half in range(2):
                        pt = ps_tr.tile([128, 2, 128], dt.bfloat16, tag="xtr")
                        for s2 in range(2):
                            s = 2 * half + s2
                            nc.tensor.transpose(
                                pt[:, s2, :], vst[:, 128 * s:128 * (s + 1)],
                                ident_bf)
                        for s2 in range(2):
                            kt_idx = 4 * m + 2 * half + s2
                            nc.scalar.copy(out=vtok[:, kt_idx, 0:64],
                                           in_=pt[:, s2, 0:64])
                            nc.scalar.copy(out=vtok[:, kt_idx, 65:129],
                                           in_=pt[:, s2, 64:128])

                def qkv_macro(m):
                    for w in range(5):
                        qkv_part(m, w)

                def attention_macro(b, mm, units):
                    q0 = 2048 * b + QM * mm
                    nkt = 4 * mm + 4
                    # scores + exp for all k-tiles of the macro; next
                    # macro's transpose/QKV PE work drains between k-tiles
                    # so the PE never idles while ACT paces the exps
                    done = 0
                    pbs = []
                    for j in range(nkt):
                        rel = j - 4 * mm
                        lo = 128 * rel if rel > 0 else 0
                        ks = slice(2048 * b + 128 * j, 2048 * b + 128 * (j + 1))
                        ps = ps_sc.tile([128, 2, QM], dt.float32, tag="sc")
                        for h in range(2):
                            hp = slice(64 * h, 64 * (h + 1))
                            nc.tensor.matmul(
                                ps[:, h, lo:QM], k_T[hp, ks],
                                q_T[hp, q0 + lo:q0 + QM],
                                start=True, stop=True)
                        pb = pr_pool.tile([128, 2, QM], dt.bfloat16, tag="pr",
                                          bufs=17)
                        if lo > 0:
                            nc.vector.memset(pb[:, :, 0:lo], 0.0)
                        nc.scalar.activation(out=pb[:, :, lo:QM],
                                             in_=ps[:, :, lo:QM],
                                             func=AF.Exp, scale=0.125)
                        if rel >= 0:
                            nc.vector.tensor_mul(
                                pb[:, :, lo:lo + 128], pb[:, :, lo:lo + 128],
                                tri2)
                        pbs.append(pb)
                        spread = max(0, len(units) - 4)
                        target = spread * (j + 1) // nkt
                        while done < target:
                            units[done]()
                            done += 1

                    # AV: one psum accumulation group per (slice, head);
                    # h0/h1 ride separate banks so both can be open at once
                    csbs = [a2a_pool.tile([128, 4, 64], dt.bfloat16,
                                          tag="csb", name=f"csb{hh}", bufs=2)
                            for hh in range(2)]
                    for s in range(4):
                        if done < len(units):
                            units[done]()
                            done += 1
                        for h in range(2):
                            pc = psctx.tile([128, 65], dt.float32, tag="ctx",
                                            name=f"pc{h}_{s}")
                            for j in range(nkt):
                                nc.tensor.matmul(
                                    pc, pbs[j][:, h, 128 * s:128 * (s + 1)],
                                    vtok[:, 16 * b + j, 65 * h:65 * (h + 1)],
                                    start=(j == 0), stop=(j == nkt - 1))
                            rec = a2a_pool.tile([128, 1], dt.float32,
                                                tag="rec", bufs=2)
                            nc.vector.reciprocal(out=rec, in_=pc[:, 64:65])
                            nc.vector.tensor_scalar_mul(
                                out=csbs[h][:, s, :], in0=pc[:, 0:64],
                                scalar1=rec)
                    while done < len(units):
                        units[done]()
                        done += 1
                    for h in range(2):
                        for half in range(2):
                            nc.sync.dma_start(
                                out=a2a_in[b][2 * mm + half, h].rearrange(
                                    "(s p) d -> p s d", p=128),
                                in_=csbs[h][:, 2 * half:2 * half + 2, :])

                def make_units(m):
                    """DVE-side LN for macro m issued eagerly; returns PE
                    closures (transposes + QKV matmuls) to drain later."""
                    units = []
                    for i, (x_t, mean, rstd) in enumerate(
                            ln_macro(x_full, QM * m)):
                        t = 4 * m + i
                        xnb = xt_pool.tile([128, D], dt.bfloat16, tag="xnb",
                                           bufs=5)
                        nc.vector.tensor_scalar(out=xnb, in0=x_t, scalar1=mean,
                                                scalar2=rstd, op0=OP.subtract,
                                                op1=OP.mult)
                        for half in range(2):
                            def u_tr(xnb=xnb, t=t, half=half):
                                pt = ps_tr.tile([128, 4, 128], dt.bfloat16,
                                                tag="xtr")
                                for s2 in range(4):
                                    kc = 4 * half + s2
                                    nc.tensor.transpose(
                                        pt[:, s2, :],
                                        xnb[:, 128 * kc:128 * (kc + 1)],
                                        ident_bf)
                                dst = xn_T[:, 4 * half:4 * half + 4,
                                           128 * t:128 * (t + 1)]
                                if half == 0:
                                    nc.scalar.copy(out=dst, in_=pt)
                                else:
                                    nc.vector.tensor_copy(out=dst, in_=pt)
                            units.append(u_tr)
                    for w in range(5):
                        units.append(lambda m=m, w=w: qkv_part(m, w))
                    return units

                for u in make_units(0):
                    u()
                for m in range(8):
                    nxt = make_units(m + 1) if m < 7 else []
                    attention_macro(m // 4, m % 4, nxt)
                    if m == 3:
                        nc.gpsimd.collective_compute(
                            "AllToAll", mybir.AluOpType.bypass,
                            replica_groups=[list(range(NCORES))],
                            ins=[a2a_in[0].opt()], outs=[a2a_out[0].opt()],
                        )
                    if m == 2:
                        nc.sync.dma_start(out=dense_sb, in_=dense_wt)
                    if m == 1:
                        # true xn (gamma/beta applied, fp32) for own shard
                        g1b = a2a_pool.tile([128, D], dt.float32, tag="g1b",
                                            bufs=1)
                        b1b = a2a_pool.tile([128, D], dt.float32, tag="b1b",
                                            bufs=1)
                        nc.sync.dma_start(out=g1b, in_=g1b_i)
                        nc.sync.dma_start(out=b1b, in_=b1b_i)
                        for i, (x_t, mean, rstd) in enumerate(
                                ln_macro(x_shard, 0)):
                            xr = xt_pool.tile([128, D], dt.float32, tag="xr",
                                              bufs=2)
                            nc.vector.tensor_scalar(out=xr, in0=x_t,
                                                    scalar1=mean,
                                                    scalar2=rstd,
                                                    op0=OP.subtract,
                                                    op1=OP.mult)
                            nc.vector.tensor_mul(xr, xr, g1b)
                            nc.vector.tensor_add(xn_sh[:, i, :], xr, b1b)

                nc.gpsimd.collective_compute(
                    "AllToAll", mybir.AluOpType.bypass,
                    replica_groups=[list(range(NCORES))],
                    ins=[a2a_in[1].opt()], outs=[a2a_out[1].opt()],
                )

            # ---------------- phase 2: dense, LN2, FFN, per batch half ----
            with ExitStack() as es2:
                P = lambda *a, **k: es2.enter_context(tc.tile_pool(*a, **k))
                fc_pool = P(name="fcw", bufs=2)
                ctx_pool = P(name="ctxT", bufs=1)
                cstg_pool = P(name="cstg", bufs=3)
                h_pool = P(name="hh", bufs=2)
                st2_pool = P(name="st2", bufs=4)
                hnT_pool = P(name="hnT", bufs=1)
                g1_pool = P(name="g1sb", bufs=1)
                prj_pool = P(name="prst", bufs=2)
                psd = P(name="psd", bufs=2, space="PSUM")
                psf = P(name="psf", bufs=2, space="PSUM")
                pse = P(name="pse", bufs=2, space="PSUM")
                out_pool = P(name="outsb", bufs=3)
                c2_pool = P(name="c2", bufs=1)

                g2b = c2_pool.tile([128, D], dt.float32)
                b2b = c2_pool.tile([128, D], dt.float32)
                fcb = c2_pool.tile([128, 32], dt.float32)
                nc.sync.dma_start(out=g2b, in_=g2b_i)
                nc.sync.dma_start(out=b2b, in_=b2b_i)
                nc.sync.dma_start(out=fcb, in_=fcb_i)

                ctxT = ctx_pool.tile([128, 8, TPC], dt.bfloat16)
                hnT = hnT_pool.tile([128, 8, TPC], dt.bfloat16)
                g1 = g1_pool.tile([128, 32, TPC], dt.bfloat16)
                hn_true = h_pool.tile([128, 4, D], dt.float32, tag="hn",
                                      bufs=1)

                def fc_dma(ch):
                    t = fc_pool.tile([128, 8, 8, 128], dt.bfloat16,
                                     tag="fcw")
                    nc.sync.dma_start(out=t, in_=fc_wt[ch])
                    return t

                def pw_dma(q4):
                    t = prj_pool.tile([128, 32, 256], dt.bfloat16, tag="pw")
                    nc.sync.dma_start(out=t, in_=proj_wt[q4])
                    return t

                def ctx_rebuild(hb):
                    for s in range(8):
                        cstg = cstg_pool.tile([128, 2, 2, 64], dt.bfloat16,
                                              tag="cs")
                        for h in range(2):
                            nc.sync.dma_start(
                                out=cstg[:, :, h, :],
                                in_=a2a_out[hb][s, h].rearrange(
                                    "(tt p) d -> p tt d", p=128))
                        pt = pse.tile([128, 2, 128], dt.bfloat16, tag="ctr")
                        for tt in range(2):
                            nc.tensor.transpose(
                                pt[:, tt, :],
                                cstg[:, tt].rearrange("p h d -> p (h d)"),
                                ident_bf)
                        dst = ctxT[:, s, 256 * hb:256 * hb + 256].rearrange(
                            "p (a b) -> p a b", a=2)
                        if s % 2 == 0:
                            nc.vector.tensor_copy(out=dst, in_=pt)
                        else:
                            nc.scalar.copy(out=dst, in_=pt)

                def dense_ln2(hb):
                    # dense: h = attn_out + (xn*g1 + b1 + dense_b)
                    h_t = h_pool.tile([128, 2, D], dt.float32, tag="ht",
                                      bufs=2)
                    for tt in range(2):
                        ts = 2 * hb + tt
                        for dh in range(2):
                            dsl = slice(512 * dh, 512 * (dh + 1))
                            ps = psd.tile([128, QM], dt.float32, tag="dn")
                            for kc in range(8):
                                nc.tensor.matmul(
                                    ps,
                                    ctxT[:, kc, 128 * ts:128 * (ts + 1)],
                                    dense_sb[:, kc, dsl],
                                    start=(kc == 0), stop=(kc == 7))
                            nc.vector.tensor_add(h_t[:, tt, dsl], ps,
                                                 xn_sh[:, ts, dsl])

                    # LN2 -> hn_true (fp32) + hnT (bf16, transposed)
                    mv2 = st2_pool.tile([128, 2, 2], dt.float32, tag="mv2")
                    for tt in range(2):
                        stats = st2_pool.tile([128, 2, 6], dt.float32,
                                              tag="bnst2")
                        nc.vector.bn_stats(out=stats[:, 0, :],
                                           in_=h_t[:, tt, 0:512])
                        nc.vector.bn_stats(out=stats[:, 1, :],
                                           in_=h_t[:, tt, 512:1024])
                        nc.vector.bn_aggr(out=mv2[:, tt, :], in_=stats)
                    rstd2 = st2_pool.tile([128, 2], dt.float32, tag="rstd2")
                    rsq2v = st2_pool.tile([128, 2], dt.float32, tag="rsq2v",
                                          bufs=2)
                    nc.vector.tensor_scalar_mul(out=rsq2v, in0=mv2[:, :, 1],
                                                scalar1=VAR_SCALE)
                    yi2 = rstd2.bitcast(dt.int32)
                    nc.vector.tensor_scalar(
                        out=yi2, in0=rsq2v.bitcast(dt.int32), scalar1=1,
                        scalar2=None, op0=OP.logical_shift_right)
                    nc.vector.tensor_scalar(
                        out=yi2, in0=yi2, scalar1=-1, scalar2=0x5f3759df,
                        op0=OP.mult, op1=OP.add)
                    t22 = st2_pool.tile([128, 2], dt.float32, tag="rsq2t",
                                        bufs=2)
                    for _ in range(2):
                        nc.vector.tensor_tensor(out=t22, in0=rstd2, in1=rstd2,
                                                op=OP.mult)
                        nc.vector.tensor_tensor(out=t22, in0=t22, in1=rsq2v,
                                                op=OP.mult)
                        nc.vector.tensor_scalar(
                            out=t22, in0=t22, scalar1=-0.5, scalar2=1.5,
                            op0=OP.mult, op1=OP.add)
                        nc.vector.tensor_tensor(out=rstd2, in0=rstd2, in1=t22,
                                                op=OP.mult)
                    for tt in range(2):
                        ts = 2 * hb + tt
                        hr = st2_pool.tile([128, D], dt.float32, tag="hr",
                                           bufs=2)
                        nc.vector.tensor_scalar(out=hr, in0=h_t[:, tt, :],
                                                scalar1=mv2[:, tt, 0:1],
                                                scalar2=rstd2[:, tt:tt + 1],
                                                op0=OP.subtract, op1=OP.mult)
                        nc.vector.tensor_mul(hn_true[:, ts, :], hr, g2b)
                        nc.vector.tensor_add(hn_true[:, ts, :],
                                             hn_true[:, ts, :], b2b)
                        hrb = st2_pool.tile([128, D], dt.bfloat16, tag="hrb",
                                            bufs=2)
                        nc.scalar.copy(out=hrb, in_=hr)
                        for half in range(2):
                            pt = pse.tile([128, 4, 128], dt.bfloat16,
                                          tag="ctr")
                            for s2 in range(4):
                                kc = 4 * half + s2
                                nc.tensor.transpose(
                                    pt[:, s2, :],
                                    hrb[:, 128 * kc:128 * (kc + 1)], ident_bf)
                            dst = hnT[:, 4 * half:4 * half + 4,
                                      128 * ts:128 * (ts + 1)]
                            if (tt + half) % 2 == 0:
                                nc.scalar.copy(out=dst, in_=pt)
                            else:
                                nc.vector.tensor_copy(out=dst, in_=pt)

                def fc_half(hb, fcw):
                    tb = slice(256 * hb, 256 * (hb + 1))
                    for ch in range(4):
                        fct = fcw[ch % 2]
                        for ht8 in range(8):
                            ht = 8 * ch + ht8
                            ps = psf.tile([128, 256], dt.float32, tag="fc")
                            for kc in range(8):
                                nc.tensor.matmul(ps, fct[:, ht8, kc, :],
                                                 hnT[:, kc, tb],
                                                 start=(kc == 0),
                                                 stop=(kc == 7))
                            nc.scalar.activation(out=g1[:, ht, tb], in_=ps,
                                                 func=AF.Gelu,
                                                 bias=fcb[:, ht:ht + 1],
                                                 scale=1.0)
                        if ch < 2:
                            fcw[ch % 2] = fc_dma(ch + 2)

                # ---- schedule: dense0, fc0, dense1, fc1, proj(merged) ----
                fcw = [fc_dma(0), fc_dma(1)]
                ctx_rebuild(0)
                dense_ln2(0)
                fc_half(0, fcw)
                # hb1 weight re-DMAs queued before the a2a#1-gated ctx DMAs
                fcw1 = [fc_dma(0), fc_dma(1)]
                pws = [pw_dma(0), pw_dma(1)]
                ctx_rebuild(1)
                dense_ln2(1)
                fc_half(1, fcw1)
                for q4 in range(4):
                    dsl = slice(256 * q4, 256 * (q4 + 1))
                    pw = pws[q4 % 2]
                    if q4 < 2:
                        pws[q4 % 2] = pw_dma(q4 + 2)
                    for ts in range(4):
                        tsl = slice(128 * ts, 128 * (ts + 1))
                        ps = psf.tile([128, 256], dt.float32, tag="fc")
                        for j in range(32):
                            nc.tensor.matmul(ps, g1[:, j, tsl], pw[:, j, :],
                                             start=(j == 0), stop=(j == 31))
                        osb = out_pool.tile([128, 256], dt.float32,
                                            tag="osb")
                        nc.vector.tensor_add(osb, ps, hn_true[:, ts, dsl])
                        nc.sync.dma_start(out=out_sh[tsl, dsl], in_=osb)

    nc.compile()
    return nc


def _np_reference(x, mask, wq_w, wq_b, wk_w, wk_b, wv_w, wv_b, dense_w,
                  dense_b, gamma1, beta1, gamma2, beta2, fc_w, proj_w):
    """Pure-numpy fallback for non-causal masks (never hit in practice)."""
    import math
    erf = np.vectorize(math.erf)

    def ln(x, g, b):
        mu = x.mean(-1, keepdims=True)
        sd = x.std(-1, ddof=1, keepdims=True)
        return g * ((x - mu) / (sd + 1e-6)) + b

    x = x.astype(np.float64)
    xn = ln(x, gamma1, beta1)
    q = (xn @ wq_w.T + wq_b).reshape(B, S, H, DEP).transpose(0, 2, 1, 3)
    k = (xn @ wk_w.T + wk_b).reshape(B, S, H, DEP).transpose(0, 2, 1, 3)
    v = (xn @ wv_w.T + wv_b).reshape(B, S, H, DEP).transpose(0, 2, 1, 3)
    sc = np.einsum("bhqd,bhkd->bhqk", q, k) / np.sqrt(DEP) + mask * -1e9
    sc = sc - sc.max(-1, keepdims=True)
    e = np.exp(sc)
    a = e / e.sum(-1, keepdims=True)
    ctx = np.einsum("bhqk,bhkd->bhqd", a, v).transpose(0, 2, 1, 3).reshape(
        B, S, D)
    h = xn + ctx @ dense_w.T + dense_b
    hn = ln(h, gamma2, beta2)
    t = hn @ fc_w.T
    g = 0.5 * t * (1.0 + erf(t / np.sqrt(2.0)))
    return (hn + g @ proj_w.T).astype(np.float32)


def kernel(**inputs):
    x = np.asarray(inputs["x"], np.float32)
    mask = np.asarray(inputs["mask"], np.float32)

    causal = np.array_equal(mask, np.triu(np.ones((S, S), np.float32), k=1))
    if not causal:
        return _np_reference(**{k: np.asarray(v, np.float64 if
                                              np.asarray(v).dtype != np.int32
                                              else np.int32)
                                for k, v in inputs.items()}).reshape(B, S, D)

    if "nc" not in _cache:
        _cache["nc"] = _build_program()
    nc = _cache["nc"]

    bf16 = ml_dtypes.bfloat16
    g1 = np.asarray(inputs["gamma1"], np.float32)
    b1 = np.asarray(inputs["beta1"], np.float32)
    g2 = np.asarray(inputs["gamma2"], np.float32)
    b2 = np.asarray(inputs["beta2"], np.float32)
    dense_w = np.asarray(inputs["dense_w"], np.float32)
    dense_b = np.asarray(inputs["dense_b"], np.float32)
    fc_w = np.asarray(inputs["fc_w"], np.float32)
    proj_w = np.asarray(inputs["proj_w"], np.float32)

    xf = x.reshape(NT, D)
    shard_rows = []
    for c in range(NCORES):
        base = 512 * (c // 2) + 256 * (c % 2)
        shard_rows.append(np.concatenate(
            [base + np.arange(256), 2048 + base + np.arange(256)]))
    bcast = lambda v: np.ascontiguousarray(
        np.broadcast_to(v.astype(np.float32), (128, D)))

    # causal diagonal-band 0/1 multiplicative mask [k2, {h0,h1}, q2]
    tri = np.ones((128, 128), np.float32)
    kk = np.arange(128)[:, None]
    qq = np.arange(128)[None, :]
    tri[kk > qq] = 0.0
    mask_tri = np.ascontiguousarray(
        np.repeat(tri[:, None, :], 2, axis=1)).astype(ml_dtypes.bfloat16)

    fc_eff = fc_w * g2[None, :]
    fcb = fc_w @ b2
    # pre-arranged, partition-contiguous weight layouts
    # dense: [p, kc, m] with contraction row = 128*kc + p
    dense_rr = np.ascontiguousarray(
        dense_w.T.reshape(8, 128, D).transpose(1, 0, 2)).astype(bf16)
    # fc: [ch, p, ht8, kc, m], weight rows = fc out (32x128), cols = D
    fc_rr = np.ascontiguousarray(
        fc_eff.reshape(4, 8, 128, 8, 128).transpose(0, 4, 1, 3, 2)).astype(
            bf16)
    # proj: [q4, p, j, dout], contraction row = 128*j + p
    proj_rr = np.ascontiguousarray(
        proj_w.T.reshape(32, 128, 4, 256).transpose(2, 1, 0, 3)).astype(bf16)
    in_maps = []
    for c in range(NCORES):
        rows = slice(128 * c, 128 * (c + 1))
        im = {
            "x_full": xf,
            "x_shard": np.ascontiguousarray(xf[shard_rows[c]]),
            "g1b": bcast(g1), "b1b": bcast(b1 + dense_b),
            "g2b": bcast(g2), "b2b": bcast(b2),
            "dense_wt": dense_rr,
            "fc_wt": fc_rr,
            "fcb": np.ascontiguousarray(fcb.reshape(32, 128).T),
            "proj_wt": proj_rr,
            "mask_tri": mask_tri,
        }
        for nm, w, bias in (("q", np.asarray(inputs["wq_w"], np.float32),
                             np.asarray(inputs["wq_b"], np.float32)),
                            ("k", np.asarray(inputs["wk_w"], np.float32),
                             np.asarray(inputs["wk_b"], np.float32)),
                            ("v", np.asarray(inputs["wv_w"], np.float32),
                             np.asarray(inputs["wv_b"], np.float32))):
            wslice = w[rows]                     # [128, D]
            im[f"w{nm}t"] = np.ascontiguousarray(
                (wslice * g1[None, :]).T.reshape(8, 128, 128).transpose(
                    1, 0, 2)).astype(bf16)
            im[f"{nm}b"] = (bias[rows] + wslice @ b1).reshape(128, 1)
        in_maps.append(im)

    global _last_in_maps
    _last_in_maps = in_maps
    from concourse import bass_utils
    res = bass_utils.run_bass_kernel_spmd(nc, in_maps,
                                          core_ids=list(range(NCORES)))
    out = np.empty((NT, D), np.float32)
    for c in range(NCORES):
        out[shard_rows[c]] = res.results[c]["out_shard"]
    return out.reshape(B, S, D)


# revision 20
# speedup vs baseline: 1.1420x; 1.1420x over previous
"""Trainium2 Bass kernel for a decoder block (LN -> MHA -> LN -> FFN).

Sharding: heads across the 8 cores for attention (2 heads/core), tokens
across cores for dense/LN2/FFN (512 tokens/core), connected by an
AllToAll of the softmax-normalized ctx in bf16 — one collective per
batch; the first overlaps batch-1 attention, the second overlaps the
batch-0 half of the dense/FFN phase.

v2 structural changes vs v1:
- AV matmul flipped: probs tiles are the stationary operand, ctx comes
  out token-major [q, 64+1] with the softmax denominator in the last
  column -> per-partition reciprocal + tensor_scalar (kills the
  lane-starved [1,512] reciprocal / broadcast-matmul / big multiply).
- A2A payload is token-major; consumer rebuilds feature-major ctxT with
  16 PE transposes per half (cheap, in a phase where PE has slack).
- Causal mask adds narrowed to the 128-wide diagonal band, both heads
  in one op; exp for both heads in one ACT instruction per k-tile.
- Causal mask applied AFTER exp as a bf16 0/1 band multiply, so the
  DVE leaves the scores->exp critical cycle.
- rstd via DVE fast-rsqrt (bit trick + 2 Newton steps) -> no ACT
  Sqrt/Exp activation-table swaps.
- All weights pre-arranged on the host into partition-contiguous
  layouts (one contiguous chunk per SBUF partition per DMA) to kill
  Sync-queue descriptor-generation cost.
- Phase 2 schedule: ctx0/dense0/LN2-0 -> fc0 -> ctx1/dense1/LN2-1 ->
  fc1 -> merged proj over all 512 tokens (proj weights loaded once);
  fc weights stream while A2A#1 is in flight.
"""

import numpy as np
import ml_dtypes

B, S, D = 2, 2048, 1024
H, DEP = 16, 64
NT = B * S            # 4096 flattened tokens
NCORES = 8
HPC = H // NCORES     # 2 heads per core
TPC = NT // NCORES    # 512 tokens per core
QM = 512              # q-macro / token-macro size
NEG = -1.0e9
AV_LAG = 3            # k-tiles of slack between exp and AV consumption

_cache = {}


def _build_program():
    from contextlib import ExitStack
    import concourse.bacc as bacc
    import concourse.tile as tile
    import concourse.mybir as mybir
    from concourse.masks import make_identity

    dt = mybir.dt
    AF = mybir.ActivationFunctionType
    OP = mybir.AluOpType

    nc = bacc.Bacc("TRN2", target_bir_lowering=False, debug=False,
                   num_devices=NCORES)

    def din(name, shape, dtype=dt.float32):
        return nc.dram_tensor(name, shape, dtype, kind="ExternalInput").ap()

    x_full = din("x_full", [NT, D])
    x_shard = din("x_shard", [TPC, D])
    wqt = din("wqt", [128, 8, 128], dt.bfloat16)
    wkt = din("wkt", [128, 8, 128], dt.bfloat16)
    wvt = din("wvt", [128, 8, 128], dt.bfloat16)
    qb_i = din("qb", [128, 1])
    kb_i = din("kb", [128, 1])
    vb_i = din("vb", [128, 1])
    g1b_i = din("g1b", [128, D])
    b1b_i = din("b1b", [128, D])
    g2b_i = din("g2b", [128, D])
    b2b_i = din("b2b", [128, D])
    dense_wt = din("dense_wt", [128, 8, D], dt.bfloat16)
    fc_wt = din("fc_wt", [4, 128, 8, 8, 128], dt.bfloat16)
    fcb_i = din("fcb", [128, 32])
    proj_wt = din("proj_wt", [4, 128, 32, 256], dt.bfloat16)
    mask_i = din("mask_tri", [128, 2, 128], dt.bfloat16)
    out_sh = nc.dram_tensor("out_shard", [TPC, D], dt.float32,
                            kind="ExternalOutput").ap()

    VAR_SCALE = float(D) / float(D - 1)   # ddof=1 correction
    EPS = 1e-6

    with tile.TileContext(nc) as tc:
        with ExitStack() as es0:
            consts = es0.enter_context(tc.tile_pool(name="consts", bufs=1))
            dram = es0.enter_context(
                tc.tile_pool(name="dram", bufs=1, space="DRAM"))
            ident_bf = consts.tile([128, 128], dt.bfloat16)
            make_identity(nc, ident_bf)
            qb = consts.tile([128, 1], dt.float32)
            kb = consts.tile([128, 1], dt.float32)
            vb = consts.tile([128, 1], dt.float32)
            nc.sync.dma_start(out=qb, in_=qb_i)
            nc.sync.dma_start(out=kb, in_=kb_i)
            nc.sync.dma_start(out=vb, in_=vb_i)
            tri2 = consts.tile([128, 2, 128], dt.bfloat16)
            nc.sync.dma_start(out=tri2, in_=mask_i)

            xnsh_pool = es0.enter_context(tc.tile_pool(name="xnsh", bufs=1))
            xn_sh = xnsh_pool.tile([128, 4, D], dt.float32)
            dense_pool = es0.enter_context(tc.tile_pool(name="densew", bufs=1))
            dense_sb = dense_pool.tile([128, 8, D], dt.bfloat16)

            # batch-mixed shards: core c owns 256 tokens of each batch,
            # a2a payload is token-major [dst, head, tok, d].
            a2a_in = [dram.tile([NCORES, HPC, 256, DEP], dt.bfloat16,
                                name=f"a2a_in{bb}") for bb in range(2)]
            a2a_out = [dram.tile([NCORES, HPC, 256, DEP], dt.bfloat16,
                                 name=f"a2a_out{bb}") for bb in range(2)]

            # ------- phase 1: LN1 + QKV + attention, per 512-token macro ---
            with ExitStack() as es1:
                P = lambda *a, **k: es1.enter_context(tc.tile_pool(*a, **k))
                xt_pool = P(name="xt", bufs=3)
                st_pool = P(name="stats", bufs=3)
                xnT_pool = P(name="xnT", bufs=1)
                qkT_pool = P(name="qkT", bufs=1)
                v_pool = P(name="vtok", bufs=1)
                wq_pool = P(name="wq", bufs=1)
                ps_tr = P(name="ps_tr", bufs=1, space="PSUM")
                ps_sc = P(name="ps_sc", bufs=2, space="PSUM")
                ps_qk = P(name="ps_qk", bufs=1, space="PSUM")
                psctx = P(name="psctx", bufs=2, space="PSUM")
                pr_pool = P(name="probs", bufs=17)
                a2a_pool = P(name="a2asb", bufs=4)

                xn_T = xnT_pool.tile([128, 8, NT], dt.bfloat16)
                q_T = qkT_pool.tile([128, NT], dt.bfloat16)
                k_T = qkT_pool.tile([128, NT], dt.bfloat16)
                vtok = v_pool.tile([128, 32, 130], dt.bfloat16)
                nc.vector.memset(vtok[:, :, 64:65], 1.0)
                nc.vector.memset(vtok[:, :, 129:130], 1.0)

                wq_sb = wq_pool.tile([128, 8, 128], dt.bfloat16)
                wk_sb = wq_pool.tile([128, 8, 128], dt.bfloat16)
                wv_sb = wq_pool.tile([128, 8, 128], dt.bfloat16)
                nc.sync.dma_start(out=wq_sb, in_=wqt)
                nc.sync.dma_start(out=wk_sb, in_=wkt)
                nc.sync.dma_start(out=wv_sb, in_=wvt)

                def rsqrt_dve(out_ap, var_ap, n):
                    """rstd = 1/sqrt(var*VAR_SCALE) entirely on DVE:
                    Quake-III bit trick + two Newton iterations."""
                    v = st_pool.tile([128, n], dt.float32, tag="rsq_v",
                                     bufs=2, name="rsq_v")
                    nc.vector.tensor_scalar_mul(out=v, in0=var_ap,
                                                scalar1=VAR_SCALE)
                    y = out_ap
                    yi = y.bitcast(dt.int32)
                    nc.vector.tensor_scalar(
                        out=yi, in0=v.bitcast(dt.int32), scalar1=1,
                        scalar2=None, op0=OP.logical_shift_right)
                    nc.vector.tensor_scalar(
                        out=yi, in0=yi, scalar1=-1, scalar2=0x5f3759df,
                        op0=OP.mult, op1=OP.add)
                    t2 = st_pool.tile([128, n], dt.float32, tag="rsq_t",
                                      bufs=2, name="rsq_t")
                    for _ in range(2):
                        nc.vector.tensor_tensor(out=t2, in0=y, in1=y,
                                                op=OP.mult)
                        nc.vector.tensor_tensor(out=t2, in0=t2, in1=v,
                                                op=OP.mult)
                        nc.vector.tensor_scalar(
                            out=t2, in0=t2, scalar1=-0.5, scalar2=1.5,
                            op0=OP.mult, op1=OP.add)
                        nc.vector.tensor_tensor(out=y, in0=y, in1=t2,
                                                op=OP.mult)

                def ln_macro(src, base_row):
                    """Stats for 4 consecutive 128-row tiles; rstd via DVE
                    fast-rsqrt. Returns [(x_t, mean, rstd)]."""
                    mv4 = st_pool.tile([128, 4, 2], dt.float32, tag="mv4")
                    xts = []
                    for i in range(4):
                        x_t = xt_pool.tile([128, D], dt.float32, tag="xt",
                                           bufs=4)
                        r0 = base_row + 128 * i
                        nc.sync.dma_start(out=x_t, in_=src[r0:r0 + 128, :])
                        stats = st_pool.tile([128, 2, 6], dt.float32,
                                             tag="bnst")
                        nc.vector.bn_stats(out=stats[:, 0, :],
                                           in_=x_t[:, 0:512])
                        nc.vector.bn_stats(out=stats[:, 1, :],
                                           in_=x_t[:, 512:1024])
                        nc.vector.bn_aggr(out=mv4[:, i, :], in_=stats)
                        xts.append(x_t)
                    rstd4 = st_pool.tile([128, 4], dt.float32, tag="rstd4")
                    rsqrt_dve(rstd4, mv4[:, :, 1], 4)
                    return [(xts[i], mv4[:, i, 0:1], rstd4[:, i:i + 1])
                            for i in range(4)]

                def qkv_part(m, which):
                    tok = slice(QM * m, QM * (m + 1))
                    if which < 2:
                        w_sb, bias, dst = ((wq_sb, qb, q_T),
                                           (wk_sb, kb, k_T))[which]
                        ps = ps_qk.tile([128, QM], dt.float32, tag="qk")
                        for kc in range(8):
                            nc.tensor.matmul(ps, w_sb[:, kc, :],
                                             xn_T[:, kc, tok],
                                             start=(kc == 0), stop=(kc == 7))
                        nc.vector.tensor_scalar_add(dst[:, tok], ps, bias)
                        return
                    ps = ps_qk.tile([128, QM], dt.float32, tag="qk")
                    for kc in range(8):
                        nc.tensor.matmul(ps, wv_sb[:, kc, :], xn_T[:, kc, tok],
                                         start=(kc == 0), stop=(kc == 7))
                    vst = a2a_pool.tile([128, QM], dt.bfloat16, tag="vst",
                                        bufs=2)
                    nc.vector.tensor_scalar_add(vst, ps, vb)
                    for half in range(2):
                        pt = ps_tr.tile([128, 2, 128], dt.bfloat16, tag="xtr")
                        for s2 in range(2):
                            s = 2 * half + s2
                            nc.tensor.transpose(
                                pt[:, s2, :], vst[:, 128 * s:128 * (s + 1)],
                                ident_bf)
                        for s2 in range(2):
                            kt_idx = 4 * m + 2 * half + s2
                            nc.scalar.copy(out=vtok[:, kt_idx, 0:64],
                                           in_=pt[:, s2, 0:64])
                            nc.scalar.copy(out=vtok[:, kt_idx, 65:129],
                                           in_=pt[:, s2, 64:128])

                def qkv_macro(m):
                    for w in range(3):
                        qkv_part(m, w)

                def attention_macro(b, mm, units):
                    q0 = 2048 * b + QM * mm
                    nkt = 4 * mm + 4
                    # scores + exp for all k-tiles of the macro; next
                    # macro's transpose/QKV PE work drains between k-tiles
                    # so the PE never idles while ACT paces the exps
                    done = 0
                    pbs = []
                    for j in range(nkt):
                        rel = j - 4 * mm
                        lo = 128 * rel if rel > 0 else 0
                        ks = slice(2048 * b + 128 * j, 2048 * b + 128 * (j + 1))
                        ps = ps_sc.tile([128, 2, QM], dt.float32, tag="sc")
                        for h in range(2):
                            hp = slice(64 * h, 64 * (h + 1))
                            nc.tensor.matmul(
                                ps[:, h, lo:QM], k_T[hp, ks],
                                q_T[hp, q0 + lo:q0 + QM],
                                start=True, stop=True)
                        pb = pr_pool.tile([128, 2, QM], dt.bfloat16, tag="pr",
                                          bufs=17)
                        if lo > 0:
                            nc.vector.memset(pb[:, :, 0:lo], 0.0)
                        nc.scalar.activation(out=pb[:, :, lo:QM],
                                             in_=ps[:, :, lo:QM],
                                             func=AF.Exp, scale=0.125)
                        if rel >= 0:
                            nc.vector.tensor_mul(
                                pb[:, :, lo:lo + 128], pb[:, :, lo:lo + 128],
                                tri2)
                        pbs.append(pb)
                        target = len(units) * (j + 1) // nkt
                        while done < target:
                            units[done]()
                            done += 1

                    # AV: one psum accumulation group per (slice, head);
                    # h0/h1 ride separate banks so both can be open at once
                    csbs = [a2a_pool.tile([128, 4, 64], dt.bfloat16,
                                          tag="csb", name=f"csb{hh}", bufs=2)
                            for hh in range(2)]
                    for s in range(4):
                        for h in range(2):
                            pc = psctx.tile([128, 65], dt.float32, tag="ctx",
                                            name=f"pc{h}_{s}")
                            for j in range(nkt):
                                nc.tensor.matmul(
                                    pc, pbs[j][:, h, 128 * s:128 * (s + 1)],
                                    vtok[:, 16 * b + j, 65 * h:65 * (h + 1)],
                                    start=(j == 0), stop=(j == nkt - 1))
                            rec = a2a_pool.tile([128, 1], dt.float32,
                                                tag="rec", bufs=2)
                            nc.vector.reciprocal(out=rec, in_=pc[:, 64:65])
                            nc.vector.tensor_scalar_mul(
                                out=csbs[h][:, s, :], in0=pc[:, 0:64],
                                scalar1=rec)
                    while done < len(units):
                        units[done]()
                        done += 1
                    for h in range(2):
                        for half in range(2):
                            nc.sync.dma_start(
                                out=a2a_in[b][2 * mm + half, h].rearrange(
                                    "(s p) d -> p s d", p=128),
                                in_=csbs[h][:, 2 * half:2 * half + 2, :])

                def make_units(m):
                    """DVE-side LN for macro m issued eagerly; returns PE
                    closures (transposes + QKV matmuls) to drain later."""
                    units = []
                    for i, (x_t, mean, rstd) in enumerate(
                            ln_macro(x_full, QM * m)):
                        t = 4 * m + i
                        xnb = xt_pool.tile([128, D], dt.bfloat16, tag="xnb",
                                           bufs=5)
                        nc.vector.tensor_scalar(out=xnb, in0=x_t, scalar1=mean,
                                                scalar2=rstd, op0=OP.subtract,
                                                op1=OP.mult)
                        for half in range(2):
                            def u_tr(xnb=xnb, t=t, half=half):
                                pt = ps_tr.tile([128, 4, 128], dt.bfloat16,
                                                tag="xtr")
                                for s2 in range(4):
                                    kc = 4 * half + s2
                                    nc.tensor.transpose(
                                        pt[:, s2, :],
                                        xnb[:, 128 * kc:128 * (kc + 1)],
                                        ident_bf)
                                dst = xn_T[:, 4 * half:4 * half + 4,
                                           128 * t:128 * (t + 1)]
                                if half == 0:
                                    nc.scalar.copy(out=dst, in_=pt)
                                else:
                                    nc.vector.tensor_copy(out=dst, in_=pt)
                            units.append(u_tr)
                    for w in range(3):
                        units.append(lambda m=m, w=w: qkv_part(m, w))
                    return units

                for u in make_units(0):
                    u()
                for m in range(8):
                    nxt = make_units(m + 1) if m < 7 else []
                    attention_macro(m // 4, m % 4, nxt)
                    if m == 3:
                        nc.gpsimd.collective_compute(
                            "AllToAll", mybir.AluOpType.bypass,
                            replica_groups=[list(range(NCORES))],
                            ins=[a2a_in[0].opt()], outs=[a2a_out[0].opt()],
                        )
                    if m == 2:
                        nc.sync.dma_start(out=dense_sb, in_=dense_wt)
                    if m == 1:
                        # true xn (gamma/beta applied, fp32) for own shard
                        g1b = a2a_pool.tile([128, D], dt.float32, tag="g1b",
                                            bufs=1)
                        b1b = a2a_pool.tile([128, D], dt.float32, tag="b1b",
                                            bufs=1)
                        nc.sync.dma_start(out=g1b, in_=g1b_i)
                        nc.sync.dma_start(out=b1b, in_=b1b_i)
                        for i, (x_t, mean, rstd) in enumerate(
                                ln_macro(x_shard, 0)):
                            xr = xt_pool.tile([128, D], dt.float32, tag="xr",
                                              bufs=2)
                            nc.vector.tensor_scalar(out=xr, in0=x_t,
                                                    scalar1=mean,
                                                    scalar2=rstd,
                                                    op0=OP.subtract,
                                                    op1=OP.mult)
                            nc.vector.tensor_mul(xr, xr, g1b)
                            nc.vector.tensor_add(xn_sh[:, i, :], xr, b1b)

                nc.gpsimd.collective_compute(
                    "AllToAll", mybir.AluOpType.bypass,
                    replica_groups=[list(range(NCORES))],
                    ins=[a2a_in[1].opt()], outs=[a2a_out[1].opt()],
                )

            # ---------------- phase 2: dense, LN2, FFN, per batch half ----
            with ExitStack() as es2:
                P = lambda *a, **k: es2.enter_context(tc.tile_pool(*a, **k))
                fc_pool = P(name="fcw", bufs=2)
                ctx_pool = P(name="ctxT", bufs=1)
                cstg_pool = P(name="cstg", bufs=3)
                h_pool = P(name="hh", bufs=2)
                st2_pool = P(name="st2", bufs=4)
                hnT_pool = P(name="hnT", bufs=1)
                g1_pool = P(name="g1sb", bufs=1)
                prj_pool = P(name="prst", bufs=2)
                psd = P(name="psd", bufs=2, space="PSUM")
                psf = P(name="psf", bufs=2, space="PSUM")
                pse = P(name="pse", bufs=2, space="PSUM")
                out_pool = P(name="outsb", bufs=3)
                c2_pool = P(name="c2", bufs=1)

                g2b = c2_pool.tile([128, D], dt.float32)
                b2b = c2_pool.tile([128, D], dt.float32)
                fcb = c2_pool.tile([128, 32], dt.float32)
                nc.sync.dma_start(out=g2b, in_=g2b_i)
                nc.sync.dma_start(out=b2b, in_=b2b_i)
                nc.sync.dma_start(out=fcb, in_=fcb_i)

                ctxT = ctx_pool.tile([128, 8, TPC], dt.bfloat16)
                hnT = hnT_pool.tile([128, 8, TPC], dt.bfloat16)
                g1 = g1_pool.tile([128, 32, TPC], dt.bfloat16)
                hn_true = h_pool.tile([128, 4, D], dt.float32, tag="hn",
                                      bufs=1)

                def fc_dma(ch):
                    t = fc_pool.tile([128, 8, 8, 128], dt.bfloat16,
                                     tag="fcw")
                    nc.sync.dma_start(out=t, in_=fc_wt[ch])
                    return t

                def pw_dma(q4):
                    t = prj_pool.tile([128, 32, 256], dt.bfloat16, tag="pw")
                    nc.sync.dma_start(out=t, in_=proj_wt[q4])
                    return t

                def ctx_rebuild(hb):
                    for s in range(8):
                        cstg = cstg_pool.tile([128, 2, 2, 64], dt.bfloat16,
                                              tag="cs")
                        for h in range(2):
                            nc.sync.dma_start(
                                out=cstg[:, :, h, :],
                                in_=a2a_out[hb][s, h].rearrange(
                                    "(tt p) d -> p tt d", p=128))
                        pt = pse.tile([128, 2, 128], dt.bfloat16, tag="ctr")
                        for tt in range(2):
                            nc.tensor.transpose(
                                pt[:, tt, :],
                                cstg[:, tt].rearrange("p h d -> p (h d)"),
                                ident_bf)
                        dst = ctxT[:, s, 256 * hb:256 * hb + 256].rearrange(
                            "p (a b) -> p a b", a=2)
                        if s % 2 == 0:
                            nc.vector.tensor_copy(out=dst, in_=pt)
                        else:
                            nc.scalar.copy(out=dst, in_=pt)

                def dense_ln2(hb):
                    # dense: h = attn_out + (xn*g1 + b1 + dense_b)
                    h_t = h_pool.tile([128, 2, D], dt.float32, tag="ht",
                                      bufs=2)
                    for tt in range(2):
                        ts = 2 * hb + tt
                        for dh in range(2):
                            dsl = slice(512 * dh, 512 * (dh + 1))
                            ps = psd.tile([128, QM], dt.float32, tag="dn")
                            for kc in range(8):
                                nc.tensor.matmul(
                                    ps,
                                    ctxT[:, kc, 128 * ts:128 * (ts + 1)],
                                    dense_sb[:, kc, dsl],
                                    start=(kc == 0), stop=(kc == 7))
                            nc.vector.tensor_add(h_t[:, tt, dsl], ps,
                                                 xn_sh[:, ts, dsl])

                    # LN2 -> hn_true (fp32) + hnT (bf16, transposed)
                    mv2 = st2_pool.tile([128, 2, 2], dt.float32, tag="mv2")
                    for tt in range(2):
                        stats = st2_pool.tile([128, 2, 6], dt.float32,
                                              tag="bnst2")
                        nc.vector.bn_stats(out=stats[:, 0, :],
                                           in_=h_t[:, tt, 0:512])
                        nc.vector.bn_stats(out=stats[:, 1, :],
                                           in_=h_t[:, tt, 512:1024])
                        nc.vector.bn_aggr(out=mv2[:, tt, :], in_=stats)
                    rstd2 = st2_pool.tile([128, 2], dt.float32, tag="rstd2")
                    rsq2v = st2_pool.tile([128, 2], dt.float32, tag="rsq2v",
                                          bufs=2)
                    nc.vector.tensor_scalar_mul(out=rsq2v, in0=mv2[:, :, 1],
                                                scalar1=VAR_SCALE)
                    yi2 = rstd2.bitcast(dt.int32)
                    nc.vector.tensor_scalar(
                        out=yi2, in0=rsq2v.bitcast(dt.int32), scalar1=1,
                        scalar2=None, op0=OP.logical_shift_right)
                    nc.vector.tensor_scalar(
                        out=yi2, in0=yi2, scalar1=-1, scalar2=0x5f3759df,
                        op0=OP.mult, op1=OP.add)
                    t22 = st2_pool.tile([128, 2], dt.float32, tag="rsq2t",
                                        bufs=2)
                    for _ in range(2):
                        nc.vector.tensor_tensor(out=t22, in0=rstd2, in1=rstd2,
                                                op=OP.mult)
                        nc.vector.tensor_tensor(out=t22, in0=t22, in1=rsq2v,
                                                op=OP.mult)
                        nc.vector.tensor_scalar(
                            out=t22, in0=t22, scalar1=-0.5, scalar2=1.5,
                            op0=OP.mult, op1=OP.add)
                        nc.vector.tensor_tensor(out=rstd2, in0=rstd2, in1=t22,
                                                op=OP.mult)
                    for tt in range(2):
                        ts = 2 * hb + tt
                        hr = st2_pool.tile([128, D], dt.float32, tag="hr",
                                           bufs=2)
                        nc.vector.tensor_scalar(out=hr, in0=h_t[:, tt, :],
                                                scalar1=mv2[:, tt, 0:1],
                                                scalar2=rstd2[:, tt:tt + 1],
                                                op0=OP.subtract, op1=OP.mult)
                        nc.vector.tensor_mul(hn_true[:, ts, :], hr, g2b)
                        nc.vector.tensor_add(hn_true[:, ts, :],
                                             hn_true[:, ts, :], b2b)
                        hrb = st2_pool.tile([128, D], dt.bfloat16, tag="hrb",
                                            bufs=2)
                        nc.scalar.copy(out=hrb, in_=hr)
                        for half in range(2):
                            pt = pse.tile([128, 4, 128], dt.bfloat16,
                                          tag="ctr")
                            for s2 in range(4):
                                kc = 4 * half + s2
                                nc.tensor.transpose(
                                    pt[:, s2, :],
                                    hrb[:, 128 * kc:128 * (kc + 1)], ident_bf)
                            dst = hnT[:, 4 * half:4 * half + 4,
                                      128 * ts:128 * (ts + 1)]
                            if (tt + half) % 2 == 0:
                                nc.scalar.copy(out=dst, in_=pt)
                            else:
                                nc.vector.tensor_copy(out=dst, in_=pt)

                def fc_half(hb, fcw):
                    tb = slice(256 * hb, 256 * (hb + 1))
                    for ch in range(4):
                        fct = fcw[ch % 2]
                        for ht8 in range(8):
                            ht = 8 * ch + ht8
                            ps = psf.tile([128, 256], dt.float32, tag="fc")
                            for kc in range(8):
                                nc.tensor.matmul(ps, fct[:, ht8, kc, :],
                                                 hnT[:, kc, tb],
                                                 start=(kc == 0),
                                                 stop=(kc == 7))
                            nc.scalar.activation(out=g1[:, ht, tb], in_=ps,
                                                 func=AF.Gelu,
                                                 bias=fcb[:, ht:ht + 1],
                                                 scale=1.0)
                        if ch < 2:
                            fcw[ch % 2] = fc_dma(ch + 2)

                # ---- schedule: dense0, fc0, dense1, fc1, proj(merged) ----
                fcw = [fc_dma(0), fc_dma(1)]
                ctx_rebuild(0)
                dense_ln2(0)
                fc_half(0, fcw)
                # hb1 weight re-DMAs queued before the a2a#1-gated ctx DMAs
                fcw1 = [fc_dma(0), fc_dma(1)]
                pws = [pw_dma(0), pw_dma(1)]
                ctx_rebuild(1)
                dense_ln2(1)
                fc_half(1, fcw1)
                for q4 in range(4):
                    dsl = slice(256 * q4, 256 * (q4 + 1))
                    pw = pws[q4 % 2]
                    if q4 < 2:
                        pws[q4 % 2] = pw_dma(q4 + 2)
                    for ts in range(4):
                        tsl = slice(128 * ts, 128 * (ts + 1))
                        ps = psf.tile([128, 256], dt.float32, tag="fc")
                        for j in range(32):
                            nc.tensor.matmul(ps, g1[:, j, tsl], pw[:, j, :],
                                             start=(j == 0), stop=(j == 31))
                        osb = out_pool.tile([128, 256], dt.float32,
                                            tag="osb")
                        nc.vector.tensor_add(osb, ps, hn_true[:, ts, dsl])
                        nc.sync.dma_start(out=out_sh[tsl, dsl], in_=osb)

    nc.compile()
    return nc


def _np_reference(x, mask, wq_w, wq_b, wk_w, wk_b, wv_w, wv_b, dense_w,
                  dense_b, gamma1, beta1, gamma2, beta2, fc_w, proj_w):
    """Pure-numpy fallback for non-causal masks (never hit in practice)."""
    import math
    erf = np.vectorize(math.erf)

    def ln(x, g, b):
        mu = x.mean(-1, keepdims=True)
        sd = x.std(-1, ddof=1, keepdims=True)
        return g * ((x - mu) / (sd + 1e-6)) + b

    x = x.astype(np.float64)
    xn = ln(x, gamma1, beta1)
    q = (xn @ wq_w.T + wq_b).reshape(B, S, H, DEP).transpose(0, 2, 1, 3)
    k = (xn @ wk_w.T + wk_b).reshape(B, S, H, DEP).transpose(0, 2, 1, 3)
    v = (xn @ wv_w.T + wv_b).reshape(B, S, H, DEP).transpose(0, 2, 1, 3)
    sc = np.einsum("bhqd,bhkd->bhqk", q, k) / np.sqrt(DEP) + mask * -1e9
    sc = sc - sc.max(-1, keepdims=True)
    e = np.exp(sc)
    a = e / e.sum(-1, keepdims=True)
    ctx = np.einsum("bhqk,bhkd->bhqd", a, v).transpose(0, 2, 1, 3).reshape(
        B, S, D)
    h = xn + ctx @ dense_w.T + dense_b
    hn = ln(h, gamma2, beta2)
    t = hn @ fc_w.T
    g = 0.5 * t * (1.0 + erf(t / np.sqrt(2.0)))
    return (hn + g @ proj_w.T).astype(np.float32)


def kernel(**inputs):
    x = np.asarray(inputs["x"], np.float32)
    mask = np.asarray(inputs["mask"], np.float32)

    causal = np.array_equal(mask, np.triu(np.ones((S, S), np.float32), k=1))
    if not causal:
        return _np_reference(**{k: np.asarray(v, np.float64 if
                                              np.asarray(v).dtype != np.int32
                                              else np.int32)
                                for k, v in inputs.items()}).reshape(B, S, D)

    if "nc" not in _cache:
        _cache["nc"] = _build_program()
    nc = _cache["nc"]

    bf16 = ml_dtypes.bfloat16
    g1 = np.asarray(inputs["gamma1"], np.float32)
    b1 = np.asarray(inputs["beta1"], np.float32)
    g2 = np.asarray(inputs["gamma2"], np.float32)
    b2 = np.asarray(inputs["beta2"], np.float32)
    dense_w = np.asarray(inputs["dense_w"], np.float32)
    dense_b = np.asarray(inputs["dense_b"], np.float32)
    fc_w = np.asarray(inputs["fc_w"], np.float32)
    proj_w = np.asarray(inputs["proj_w"], np.float32)

    xf = x.reshape(NT, D)
    shard_rows = []
    for c in range(NCORES):
        base = 512 * (c // 2) + 256 * (c % 2)
        shard_rows.append(np.concatenate(
            [base + np.arange(256), 2048 + base + np.arange(256)]))
    bcast = lambda v: np.ascontiguousarray(
        np.broadcast_to(v.astype(np.float32), (128, D)))

    # causal diagonal-band 0/1 multiplicative mask [k2, {h0,h1}, q2]
    tri = np.ones((128, 128), np.float32)
    kk = np.arange(128)[:, None]
    qq = np.arange(128)[None, :]
    tri[kk > qq] = 0.0
    mask_tri = np.ascontiguousarray(
        np.repeat(tri[:, None, :], 2, axis=1)).astype(ml_dtypes.bfloat16)

    fc_eff = fc_w * g2[None, :]
    fcb = fc_w @ b2
    # pre-arranged, partition-contiguous weight layouts
    # dense: [p, kc, m] with contraction row = 128*kc + p
    dense_rr = np.ascontiguousarray(
        dense_w.T.reshape(8, 128, D).transpose(1, 0, 2)).astype(bf16)
    # fc: [ch, p, ht8, kc, m], weight rows = fc out (32x128), cols = D
    fc_rr = np.ascontiguousarray(
        fc_eff.reshape(4, 8, 128, 8, 128).transpose(0, 4, 1, 3, 2)).astype(
            bf16)
    # proj: [q4, p, j, dout], contraction row = 128*j + p
    proj_rr = np.ascontiguousarray(
        proj_w.T.reshape(32, 128, 4, 256).transpose(2, 1, 0, 3)).astype(bf16)
    in_maps = []
    for c in range(NCORES):
        rows = slice(128 * c, 128 * (c + 1))
        im = {
            "x_full": xf,
            "x_shard": np.ascontiguousarray(xf[shard_rows[c]]),
            "g1b": bcast(g1), "b1b": bcast(b1 + dense_b),
            "g2b": bcast(g2), "b2b": bcast(b2),
            "dense_wt": dense_rr,
            "fc_wt": fc_rr,
            "fcb": np.ascontiguousarray(fcb.reshape(32, 128).T),
            "proj_wt": proj_rr,
            "mask_tri": mask_tri,
        }
        for nm, w, bias in (("q", np.asarray(inputs["wq_w"], np.float32),
                             np.asarray(inputs["wq_b"], np.float32)),
                            ("k", np.asarray(inputs["wk_w"], np.float32),
                             np.asarray(inputs["wk_b"], np.float32)),
                            ("v", np.asarray(inputs["wv_w"], np.float32),
                             np.asarray(inputs["wv_b"], np.float32))):
            wslice = w[rows]                     # [128, D]
            im[f"w{nm}t"] = np.ascontiguousarray(
                (wslice * g1[None, :]).T.reshape(8, 128, 128).transpose(
                    1, 0, 2)).astype(bf16)
            im[f"{nm}b"] = (bias[rows] + wslice @ b1).reshape(128, 1)
        in_maps.append(im)

    global _last_in_maps
    _last_in_maps = in_maps
    from concourse import bass_utils
    res = bass_utils.run_bass_kernel_spmd(nc, in_maps,
                                          core_ids=list(range(NCORES)))
    out = np.empty((NT, D), np.float32)
    for c in range(NCORES):
        out[shard_rows[c]] = res.results[c]["out_shard"]
    return out.reshape(B, S, D)
